# revision 20
# baseline (speedup 1.0000x reference)
"""Trainium2 Bass kernel for MHA cross-attention (nn_MHACross).

Sharding: 8 cores = 2 batches x 4 head-groups (2 heads each).
Per core (batch b, head group g):
    k,v = xmel[b] @ Wkv[g].T ; q = x[b] @ Wq[g].T ; RoPE(q, k) (scale folded
    into host-side cos/sin tables); per head scores^T = k_r @ q_r^T;
    p = exp(scores) (scores O(6), safe without max subtraction);
    out2 = v^T @ p and Z = ones^T @ (pairwise-summed p) on the PE;
    normalize by 1/Z (reciprocal + gpsimd partition broadcast, no DRAM
    bounce); y_partial = attn @ Wout[:, g].T.  Host sums 4 partials/batch.

Schedule notes:
  - KV projection runs FIRST (xmel streamed chunk-major, per-kt pieces)
    so the PE starts ~1us in and stays dense; q proj follows; attention
    last.  This keeps the HAM clock warm (no >3us PE gaps).
  - kv-proj matmuls are emitted kt-outer across (head x 512-chunk) PSUM
    regions so each arriving DMA piece immediately feeds the PE.
  - RoPE runs entirely on the vector engine (half-slice muls), the
    scalar engine only does exp + v-tile PSUM->SBUF copies.
  - Z matmul stream is halved: p tiles are pair-summed on the DVE (bf16
    2x mode), the ones-matmul reduces 13 instead of 24 tiles per
    (head, pair).
"""
import sys
sys.path.insert(0, '/opt/trn_rl_repo')
import numpy as np

DIM = 1024
NHEADS = 8
HD = 128          # head dim
HPC = 2           # heads per core
NG = 4            # head groups (cores per batch)
B, T, S = 2, 2048, 3000
NKT = DIM // 128  # contraction tiles
ROPE_BASE = 10000.0
CW = 512          # T-chunk width
PAIR = 2 * CW     # paired chunk width

_cache = {}


def _ceil_div(a, b):
    return (a + b - 1) // b


def build_nc(T=T, S=S):
    from concourse import bacc, mybir
    from concourse.tile import TileContext

    f32 = mybir.dt.float32
    bf16 = mybir.dt.bfloat16

    nc = bacc.Bacc("TRN2", target_bir_lowering=False, debug=False, num_devices=8)

    xT = nc.dram_tensor("xT", [DIM, T], bf16, kind="ExternalInput")
    xmelT = nc.dram_tensor("xmelT", [DIM, S], bf16, kind="ExternalInput")
    WqT = nc.dram_tensor("WqT", [128, NKT * HPC * HD], bf16, kind="ExternalInput")
    WkT = nc.dram_tensor("WkT", [128, NKT * HPC * HD], bf16, kind="ExternalInput")
    WvT = nc.dram_tensor("WvT", [128, NKT * HPC * HD], bf16, kind="ExternalInput")
    WoT = nc.dram_tensor("WoT", [HPC * HD, DIM], bf16, kind="ExternalInput")
    cosq = nc.dram_tensor("cosq", [HD, T], bf16, kind="ExternalInput")
    sinq = nc.dram_tensor("sinq", [HD, T], bf16, kind="ExternalInput")
    cosk = nc.dram_tensor("cosk", [HD, S], bf16, kind="ExternalInput")
    sink = nc.dram_tensor("sink", [HD, S], bf16, kind="ExternalInput")
    y = nc.dram_tensor("y", [T, DIM], f32, kind="ExternalOutput")

    n_st = _ceil_div(S, 128)
    n_tc = _ceil_div(T, CW)
    # big chunks (<=1024 wide) for DMA/proj/rope granularity
    s_big = [(0, 512), (512, 512), (1024, 1024), (2048, S - 2048)]
    t_big = [(i * PAIR, min(PAIR, T - i * PAIR)) for i in range(_ceil_div(T, PAIR))]
    t_chunks = [(i * CW, min(CW, T - i * CW)) for i in range(n_tc)]
    t_pairs = [t_chunks[i:i + 2] for i in range(0, n_tc, 2)]
    # Z reduction chunks: full-128 st tiles group in fours (DVE-summed),
    # ragged tail tiles stay single for the ones-matmul
    n_full = S // 128            # 23
    z_chunks = [list(range(a, min(a + 4, n_full))) for a in range(0, n_full, 4)]
    z_singles = list(range(n_full, n_st))        # [23]
    z_chunk_of_last = {c[-1]: i for i, c in enumerate(z_chunks)}

    with TileContext(nc) as tc:
        with tc.tile_pool(name="wpool", bufs=1) as wp, \
             tc.tile_pool(name="persist", bufs=1) as pp:
            # persistent weights
            wq = wp.tile([128, NKT, HPC * HD], bf16)
            wk = wp.tile([128, NKT, HPC * HD], bf16)
            wv = wp.tile([128, NKT, HPC * HD], bf16)
            wo = []
            for h in range(HPC):
                wo_h = wp.tile([128, DIM], bf16, name=f"wo{h}", uniquify=True)
                wo.append(wo_h)
            ones = wp.tile([128, 1], bf16)
            nc.vector.memset(ones[:], 1.0)
            onesrow = wp.tile([1, 128], bf16)
            nc.vector.memset(onesrow[:], 1.0)

            # persistent activations
            kT_r = [pp.tile([128, S], bf16, name=f"kT{h}", uniquify=True) for h in range(HPC)]
            qT_r = [pp.tile([128, T], bf16, name=f"qT{h}", uniquify=True) for h in range(HPC)]
            v_sb = pp.tile([128, n_st, HPC * HD], bf16)

            with tc.tile_pool(name="csP", bufs=6) as csp, \
                 tc.tile_pool(name="aoP", bufs=2 * HPC + 2) as aoP, \
                 tc.tile_pool(name="zP", bufs=4) as zP, \
                 tc.tile_pool(name="yP", bufs=2) as yP, \
                 tc.tile_pool(name="psA", bufs=2, space="PSUM") as psA, \
                 tc.tile_pool(name="psB", bufs=2, space="PSUM") as psB, \
                 tc.tile_pool(name="psC", bufs=2, space="PSUM") as psC:
                # weights stream on the gpsimd queue, per-kt pieces so the
                # first matmul only waits for its own 64KB
                for kt in range(NKT):
                    nc.gpsimd.dma_start(out=wk[:, kt, :], in_=WkT[:, kt * 256:(kt + 1) * 256])
                for kt in range(NKT):
                    nc.gpsimd.dma_start(out=wv[:, kt, :], in_=WvT[:, kt * 256:(kt + 1) * 256])
                for kt in range(NKT):
                    nc.gpsimd.dma_start(out=wq[:, kt, :], in_=WqT[:, kt * 256:(kt + 1) * 256])
                for h in range(HPC):
                    nc.gpsimd.dma_start(out=wo[h][:], in_=WoT[h * HD:(h + 1) * HD, :])

                def rope_phase1(ps, cs, out_sl, w, swp_pool):
                    # rotate-copies on ACT + the single DVE op that reads the
                    # PSUM tile; after this the PSUM slot is free
                    swp = swp_pool.tile([128, PAIR], f32, name="swp", tag="rt", bufs=4)
                    nc.scalar.copy(swp[0:64, :w], ps[64:128, :w])
                    nc.scalar.copy(swp[64:128, :w], ps[0:64, :w])
                    nc.vector.tensor_mul(out_sl, ps[:, :w], cs[:, :w])
                    return swp

                def rope_phase2(swp, sn, out_sl, w):
                    nc.vector.tensor_mul(swp[:, :w], swp[:, :w], sn[:, :w])
                    nc.vector.tensor_add(out_sl, out_sl, swp[:, :w])

                # ------------- projections (kv first, then q) -------------
                with tc.tile_pool(name="xpool", bufs=1) as xp, \
                     tc.tile_pool(name="rtP", bufs=3) as rtp:
                    xm = [xp.tile([128, S], bf16, name=f"xm{kt}", uniquify=True)
                          for kt in range(NKT)]
                    xq = [xp.tile([128, T], bf16, name=f"xq{kt}", uniquify=True)
                          for kt in range(NKT)]

                    def emit_qcp(c0, cw):
                        cs_sb = csp.tile([128, PAIR], bf16, name="cos_sb", tag="cos", bufs=3)
                        sn_sb = csp.tile([128, PAIR], bf16, name="sin_sb", tag="sin", bufs=3)
                        nc.sync.dma_start(out=cs_sb[:, :cw], in_=cosq[:, c0:c0 + cw])
                        nc.sync.dma_start(out=sn_sb[:, :cw], in_=sinq[:, c0:c0 + cw])
                        for kt in range(NKT):
                            nc.sync.dma_start(out=xq[kt][:, c0:c0 + cw],
                                              in_=xT[kt * 128:(kt + 1) * 128, c0:c0 + cw])
                        qps = [psA.tile([128, PAIR], f32, name="qps", tag="sc", bufs=2)
                               for _ in range(HPC)]
                        halves = [(o, min(CW, cw - o)) for o in range(0, cw, CW)]
                        for kt in range(NKT):
                            for h in range(HPC):
                                for (o, ow) in halves:
                                    nc.tensor.matmul(
                                        qps[h][:, o:o + ow],
                                        wq[:, kt, h * HD:(h + 1) * HD],
                                        xq[kt][:, c0 + o:c0 + o + ow],
                                        start=(kt == 0), stop=(kt == NKT - 1),
                                        skip_group_check=True)
                        swps = [rope_phase1(qps[h], cs_sb, qT_r[h][:, c0:c0 + cw], cw, rtp)
                                for h in range(HPC)]
                        for h in range(HPC):
                            rope_phase2(swps[h], sn_sb, qT_r[h][:, c0:c0 + cw], cw)

                    def emit_scp(s0, sw):
                        cs_sb = csp.tile([128, PAIR], bf16, name="cos_sb", tag="cos", bufs=3)
                        sn_sb = csp.tile([128, PAIR], bf16, name="sin_sb", tag="sin", bufs=3)
                        nc.sync.dma_start(out=cs_sb[:, :sw], in_=cosk[:, s0:s0 + sw])
                        nc.sync.dma_start(out=sn_sb[:, :sw], in_=sink[:, s0:s0 + sw])
                        # chunk-major DMA: all kt pieces of this column range
                        for kt in range(NKT):
                            nc.sync.dma_start(out=xm[kt][:, s0:s0 + sw],
                                              in_=xmelT[kt * 128:(kt + 1) * 128, s0:s0 + sw])
                        # k proj, kt-outer so the PE starts on the first piece
                        kps = [psA.tile([128, PAIR], f32, name="kps", tag="sc", bufs=2)
                               for _ in range(HPC)]
                        halves = [(o, min(CW, sw - o)) for o in range(0, sw, CW)]
                        for kt in range(NKT):
                            for h in range(HPC):
                                for (o, ow) in halves:
                                    nc.tensor.matmul(
                                        kps[h][:, o:o + ow],
                                        wk[:, kt, h * HD:(h + 1) * HD],
                                        xm[kt][:, s0 + o:s0 + o + ow],
                                        start=(kt == 0), stop=(kt == NKT - 1),
                                        skip_group_check=True)
                        swps = [rope_phase1(kps[h], cs_sb, kT_r[h][:, s0:s0 + sw], sw, rtp)
                                for h in range(HPC)]
                        for h in range(HPC):
                            rope_phase2(swps[h], sn_sb, kT_r[h][:, s0:s0 + sw], sw)
                        # v proj for the st tiles of this column range
                        for st in range(s0 // 128, _ceil_div(s0 + sw, 128)):
                            t0 = st * 128
                            scnt = min(128, S - t0)
                            vps = psB.tile([128, HPC * HD], f32, name="vps", tag="acc", bufs=2)
                            for kt in range(NKT):
                                nc.tensor.matmul(
                                    vps[:scnt, :],
                                    xm[kt][:, t0:t0 + scnt],
                                    wv[:, kt, :],
                                    start=(kt == 0), stop=(kt == NKT - 1))
                            nc.scalar.copy(v_sb[:scnt, st, :], vps[:scnt, :])

                    emit_scp(*s_big[0])
                    emit_scp(*s_big[1])
                    emit_qcp(*t_big[0])
                    emit_scp(*s_big[2])
                    emit_qcp(*t_big[1])
                    emit_scp(*s_big[3])

                # ---------------- attention + out projection ----------------
                with tc.tile_pool(name="pP", bufs=n_st + 10) as pP, \
                     tc.tile_pool(name="ppairP", bufs=18) as ppP:
                    G = (n_st + 3) // 4   # pipeline group size (4 groups/block)
                    groups = []
                    for pi in range(len(t_pairs)):
                        for h in range(HPC):
                            for g0 in range(0, n_st, G):
                                groups.append((pi, h, g0, min(G, n_st - g0)))
                    DELAY = 4
                    blocks = {}

                    def emit_sc_exp(key):
                        pi, h, g0, gc = key
                        pair = t_pairs[pi]
                        pw = sum(cw for _, cw in pair)
                        bk = blocks.setdefault((pi, h), {"ptiles": {}, "pairs": {}})
                        for st in range(g0, g0 + gc):
                            s0 = st * 128
                            scnt = min(128, S - s0)
                            scps = psA.tile([128, PAIR], f32, name="scps", tag="sc", bufs=2)
                            for ci, (c0, cw) in enumerate(pair):
                                nc.tensor.matmul(
                                    scps[:scnt, ci * CW: ci * CW + cw],
                                    kT_r[h][:, s0:s0 + scnt],
                                    qT_r[h][:, c0:c0 + cw],
                                    start=True, stop=True,
                                    skip_group_check=True)
                            p_t = pP.tile([128, PAIR], bf16, name="p_t", tag="p", bufs=n_st + 10)
                            nc.scalar.activation(p_t[:scnt, :pw], scps[:scnt, :pw],
                                                 mybir.ActivationFunctionType.Exp)
                            bk["ptiles"][st] = (p_t, scnt)
                            # chunked p sums (DVE, bf16 2x) feed the Z matmuls
                            if st < n_full:
                                if st % 4 == 0:
                                    bk["zacc"] = p_t
                                else:
                                    pp_t = ppP.tile([128, PAIR], bf16, name="pp_t", tag="pp", bufs=10)
                                    nc.vector.tensor_add(pp_t[:, :pw], bk["zacc"][:, :pw],
                                                         p_t[:, :pw])
                                    bk["zacc"] = pp_t
                                if st in z_chunk_of_last:
                                    bk["pairs"][z_chunk_of_last[st]] = bk["zacc"]

                    def emit_zav(key):
                        pi, h, g0, gc = key
                        pair = t_pairs[pi]
                        bk = blocks[(pi, h)]
                        last = (g0 + gc == n_st)
                        if g0 == 0:
                            # ci=0 lives at partition 0, ci=1 at partition 32
                            # (matmul out base partition must be 0/32/64)
                            bk["zps"] = psC.tile([64, CW], f32, name="zps", tag="z", bufs=2)
                            bk["o2"] = [psB.tile([128, CW], f32, name="o2ps", tag="acc", bufs=2)
                                        for _ in pair]
                        # Z matmuls on chunk-summed tiles (+ ragged singles at the end)
                        sts = list(range(g0, g0 + gc))
                        for cki, ck in enumerate(z_chunks):
                            if ck[-1] in sts:
                                pp_t = bk["pairs"][cki]
                                first = (cki == 0)
                                for ci, (c0, cw) in enumerate(pair):
                                    nc.tensor.matmul(
                                        bk["zps"][32 * ci:32 * ci + 1, :cw],
                                        ones[:128, :],
                                        pp_t[:, ci * CW: ci * CW + cw],
                                        start=first, stop=False,
                                        skip_group_check=True)
                        if last:
                            for si, st in enumerate(z_singles):
                                p_t, scnt = bk["ptiles"][st]
                                stop = (si == len(z_singles) - 1)
                                for ci, (c0, cw) in enumerate(pair):
                                    nc.tensor.matmul(
                                        bk["zps"][32 * ci:32 * ci + 1, :cw],
                                        ones[:scnt, :],
                                        p_t[:scnt, ci * CW: ci * CW + cw],
                                        start=False, stop=stop,
                                        skip_group_check=True)
                            # stage Z rows to SBUF as bf16 (single cast-copy
                            # each); broadcast + reciprocal happen later
                            bk["zrb"] = []
                            for ci, (c0, cw) in enumerate(pair):
                                zrb = zP.tile([1, CW], bf16, name="zrb", tag="zrb", bufs=4)
                                nc.vector.tensor_copy(zrb[:, :], bk["zps"][32 * ci:32 * ci + 1, :])
                                bk["zrb"].append(zrb)
                        for st in sts:
                            p_t, scnt = bk["ptiles"][st]
                            for ci, (c0, cw) in enumerate(pair):
                                nc.tensor.matmul(
                                    bk["o2"][ci][:, :cw],
                                    v_sb[:scnt, st, h * HD:(h + 1) * HD],
                                    p_t[:scnt, ci * CW: ci * CW + cw],
                                    start=(st == 0), stop=(st == n_st - 1))
                        if last:
                            # broadcast 1/Z across partitions with a ones-column
                            # matmul (no DMA), stage via ACT, then normalize
                            bk["ao"] = []
                            for ci, (c0, cw) in enumerate(pair):
                                zrp = psC.tile([128, CW], f32, name="zrp", tag="z", bufs=2)
                                nc.tensor.matmul(zrp[:, :], onesrow[:, :], bk["zrb"][ci][:, :],
                                                 start=True, stop=True,
                                                 skip_group_check=True)
                                zrep = zP.tile([128, CW], f32, name="zrep", tag="zrep", bufs=4)
                                nc.vector.reciprocal_approx_fast(out=zrep[:, :], in_=zrp[:, :])
                                ao_h = aoP.tile([128, CW], bf16, name="ao", tag="ao", bufs=2 * HPC + 2)
                                nc.vector.tensor_mul(ao_h[:, :cw], bk["o2"][ci][:, :cw],
                                                     zrep[:, :cw])
                                bk["ao"].append(ao_h)

                    def emit_outproj(pi):
                        pair = t_pairs[pi]
                        for ci, (c0, cw) in enumerate(pair):
                            for tt in range(cw // 128):
                                y_sb = yP.tile([128, DIM], f32, name="y_sb", tag="ysb", bufs=2)
                                for nn in range(DIM // 512):
                                    yps = psA.tile([128, 512], f32, name="yps", tag="sc", bufs=2)
                                    for h in range(HPC):
                                        nc.tensor.matmul(
                                            yps[:],
                                            blocks[(pi, h)]["ao"][ci][:, tt * 128:(tt + 1) * 128],
                                            wo[h][:, nn * 512:(nn + 1) * 512],
                                            start=(h == 0), stop=(h == HPC - 1))
                                    nc.vector.tensor_copy(y_sb[:, nn * 512:(nn + 1) * 512], yps[:])
                                nc.sync.dma_start(out=y[c0 + tt * 128: c0 + (tt + 1) * 128, :], in_=y_sb[:])

                    # outproj(pi) is delayed one pipeline slot past its
                    # block-last zav so the 1/Z bounce latency hides under the
                    # next group's matmuls
                    op_after = {}
                    for idx, (pi, h, g0, gc) in enumerate(groups):
                        if g0 + gc == n_st and h == HPC - 1:
                            op_after[idx + 1 if idx + 1 < len(groups) else -1] = pi

                    def do_finish(j):
                        emit_zav(groups[j])
                        if j in op_after:
                            emit_outproj(op_after[j])

                    for i, key in enumerate(groups):
                        emit_sc_exp(key)
                        if i >= DELAY:
                            do_finish(i - DELAY)
                    for j in range(max(0, len(groups) - DELAY), len(groups)):
                        do_finish(j)
                    if -1 in op_after:
                        emit_outproj(op_after[-1])

    nc.compile()
    return nc


def _host_tables(T=T, S=S):
    scale = float(HD) ** (-0.25)
    inv = 1.0 / (ROPE_BASE ** (np.arange(0, HD, 2, dtype=np.float64) / HD))  # [64]

    def tables(L):
        fr = np.outer(inv, np.arange(L, dtype=np.float64))  # [64, L]
        c = np.cos(fr) * scale
        s = np.sin(fr) * scale
        import ml_dtypes
        cos = np.concatenate([c, c], axis=0).astype(ml_dtypes.bfloat16)
        sin = np.concatenate([-s, s], axis=0).astype(ml_dtypes.bfloat16)
        return np.ascontiguousarray(cos), np.ascontiguousarray(sin)

    cosq_, sinq_ = tables(T)
    cosk_, sink_ = tables(S)
    return cosq_, sinq_, cosk_, sink_


def make_in_maps(x, xmel, Wq, Wkv, Wout):
    import ml_dtypes
    bf = ml_dtypes.bfloat16
    Bx, Tx, C = x.shape
    Sx = xmel.shape[1]
    cosq_, sinq_, cosk_, sink_ = _host_tables(Tx, Sx)

    x = np.asarray(x, dtype=np.float32)
    xmel = np.asarray(xmel, dtype=np.float32)
    Wq = np.asarray(Wq, dtype=np.float32)
    Wkv = np.asarray(Wkv, dtype=np.float32)
    Wout = np.asarray(Wout, dtype=np.float32)

    xT_b = [np.ascontiguousarray(x[b].T).astype(bf) for b in range(Bx)]
    xmelT_b = [np.ascontiguousarray(xmel[b].T).astype(bf) for b in range(Bx)]
    gsz = HPC * HD  # 256
    WqT_g, WkT_g, WvT_g, WoT_g = [], [], [], []
    for g in range(NG):
        r0 = g * gsz
        def prearr(wt):  # [DIM, gsz] -> [128, NKT*gsz], row p holds [kt, n]
            return np.ascontiguousarray(
                wt.reshape(NKT, 128, gsz).transpose(1, 0, 2).reshape(128, NKT * gsz)).astype(bf)
        WqT_g.append(prearr(Wq[r0:r0 + gsz, :].T))
        WkT_g.append(prearr(Wkv[r0:r0 + gsz, :].T))
        WvT_g.append(prearr(Wkv[DIM + r0:DIM + r0 + gsz, :].T))
        WoT_g.append(np.ascontiguousarray(Wout[:, r0:r0 + gsz].T).astype(bf))

    in_maps = []
    for c in range(Bx * NG):
        b, g = c // NG, c % NG
        in_maps.append({
            "xT": xT_b[b], "xmelT": xmelT_b[b],
            "WqT": WqT_g[g], "WkT": WkT_g[g], "WvT": WvT_g[g], "WoT": WoT_g[g],
            "cosq": cosq_, "sinq": sinq_, "cosk": cosk_, "sink": sink_,
        })
    return in_maps


def kernel(x, xmel, Wq, Wkv, Wout):
    from concourse.bass_utils import run_bass_kernel_spmd

    x = np.asarray(x, dtype=np.float32)
    xmel = np.asarray(xmel, dtype=np.float32)
    Bx, Tx, C = x.shape
    Sx = xmel.shape[1]
    assert (Bx, Tx, C, Sx) == (B, T, DIM, S)

    if "nc" not in _cache:
        _cache["nc"] = build_nc()
    nc = _cache["nc"]

    in_maps = make_in_maps(x, xmel,
                           np.asarray(Wq, dtype=np.float32),
                           np.asarray(Wkv, dtype=np.float32),
                           np.asarray(Wout, dtype=np.float32))
    res = run_bass_kernel_spmd(nc, in_maps, list(range(8)))
    out = np.zeros((B, T, DIM), dtype=np.float32)
    for c in range(8):
        b = c // NG
        out[b] += np.asarray(res.results[c]["y"], dtype=np.float32)
    return out


# revision 21
# speedup vs baseline: 1.0376x; 1.0376x over previous
"""Trainium2 Bass kernel for MHA cross-attention (nn_MHACross).

Sharding: 8 cores = 2 batches x 4 head-groups (2 heads each).
Per core (batch b, head group g):
    k,v = xmel[b] @ Wkv[g].T ; q = x[b] @ Wq[g].T ; RoPE(q, k) (scale folded
    into host-side cos/sin tables); per head scores^T = k_r @ q_r^T;
    p = exp(scores) (scores O(6), safe without max subtraction);
    out2 = v^T @ p and Z = ones^T @ (pairwise-summed p) on the PE;
    normalize by 1/Z (reciprocal + gpsimd partition broadcast, no DRAM
    bounce); y_partial = attn @ Wout[:, g].T.  Host sums 4 partials/batch.

Schedule notes:
  - KV projection runs FIRST (xmel streamed chunk-major, per-kt pieces)
    so the PE starts ~1us in and stays dense; q proj follows; attention
    last.  This keeps the HAM clock warm (no >3us PE gaps).
  - kv-proj matmuls are emitted kt-outer across (head x 512-chunk) PSUM
    regions so each arriving DMA piece immediately feeds the PE.
  - RoPE runs entirely on the vector engine (half-slice muls), the
    scalar engine only does exp + v-tile PSUM->SBUF copies.
  - Z matmul stream is halved: p tiles are pair-summed on the DVE (bf16
    2x mode), the ones-matmul reduces 13 instead of 24 tiles per
    (head, pair).
"""
import sys
sys.path.insert(0, '/opt/trn_rl_repo')
import numpy as np

DIM = 1024
NHEADS = 8
HD = 128          # head dim
HPC = 2           # heads per core
NG = 4            # head groups (cores per batch)
B, T, S = 2, 2048, 3000
NKT = DIM // 128  # contraction tiles
ROPE_BASE = 10000.0
CW = 512          # T-chunk width
PAIR = 2 * CW     # paired chunk width

_cache = {}


def _ceil_div(a, b):
    return (a + b - 1) // b


def build_nc(T=T, S=S):
    from concourse import bacc, mybir
    from concourse.tile import TileContext

    f32 = mybir.dt.float32
    bf16 = mybir.dt.bfloat16

    nc = bacc.Bacc("TRN2", target_bir_lowering=False, debug=False, num_devices=8)

    xT = nc.dram_tensor("xT", [DIM, T], bf16, kind="ExternalInput")
    xmelT = nc.dram_tensor("xmelT", [DIM, S], bf16, kind="ExternalInput")
    WqT = nc.dram_tensor("WqT", [128, NKT * HPC * HD], bf16, kind="ExternalInput")
    WkT = nc.dram_tensor("WkT", [128, NKT * HPC * HD], bf16, kind="ExternalInput")
    WvT = nc.dram_tensor("WvT", [128, NKT * HPC * HD], bf16, kind="ExternalInput")
    WoT = nc.dram_tensor("WoT", [HPC * HD, DIM], bf16, kind="ExternalInput")
    cosq = nc.dram_tensor("cosq", [HD, T], bf16, kind="ExternalInput")
    sinq = nc.dram_tensor("sinq", [HD, T], bf16, kind="ExternalInput")
    cosk = nc.dram_tensor("cosk", [HD, S], bf16, kind="ExternalInput")
    sink = nc.dram_tensor("sink", [HD, S], bf16, kind="ExternalInput")
    y = nc.dram_tensor("y", [T, DIM], f32, kind="ExternalOutput")

    n_st = _ceil_div(S, 128)
    n_tc = _ceil_div(T, CW)
    # big chunks (<=1024 wide) for DMA/proj/rope granularity
    s_big = [(i * PAIR, min(PAIR, S - i * PAIR)) for i in range(_ceil_div(S, PAIR))]
    t_big = [(i * PAIR, min(PAIR, T - i * PAIR)) for i in range(_ceil_div(T, PAIR))]
    t_chunks = [(i * CW, min(CW, T - i * CW)) for i in range(n_tc)]
    t_pairs = [t_chunks[i:i + 2] for i in range(0, n_tc, 2)]
    # Z reduction chunks: full-128 st tiles group in fours (DVE-summed),
    # ragged tail tiles stay single for the ones-matmul
    n_full = S // 128            # 23
    z_chunks = [list(range(a, min(a + 4, n_full))) for a in range(0, n_full, 4)]
    z_singles = list(range(n_full, n_st))        # [23]
    z_chunk_of_last = {c[-1]: i for i, c in enumerate(z_chunks)}

    with TileContext(nc) as tc:
        with tc.tile_pool(name="wpool", bufs=1) as wp, \
             tc.tile_pool(name="persist", bufs=1) as pp:
            # persistent weights
            wq = wp.tile([128, NKT, HPC * HD], bf16)
            wk = wp.tile([128, NKT, HPC * HD], bf16)
            wv = wp.tile([128, NKT, HPC * HD], bf16)
            wo = []
            for h in range(HPC):
                wo_h = wp.tile([128, DIM], bf16, name=f"wo{h}", uniquify=True)
                wo.append(wo_h)
            ones = wp.tile([128, 1], bf16)
            nc.vector.memset(ones[:], 1.0)
            onesrow = wp.tile([1, 128], bf16)
            nc.vector.memset(onesrow[:], 1.0)

            # persistent activations
            kT_r = [pp.tile([128, S], bf16, name=f"kT{h}", uniquify=True) for h in range(HPC)]
            qT_r = [pp.tile([128, T], bf16, name=f"qT{h}", uniquify=True) for h in range(HPC)]
            v_sb = pp.tile([128, n_st, HPC * HD], bf16)

            with tc.tile_pool(name="csP", bufs=6) as csp, \
                 tc.tile_pool(name="aoP", bufs=2 * HPC + 2) as aoP, \
                 tc.tile_pool(name="zP", bufs=4) as zP, \
                 tc.tile_pool(name="yP", bufs=2) as yP, \
                 tc.tile_pool(name="psA", bufs=2, space="PSUM") as psA, \
                 tc.tile_pool(name="psB", bufs=2, space="PSUM") as psB, \
                 tc.tile_pool(name="psC", bufs=2, space="PSUM") as psC:
                # weights stream on the gpsimd queue, per-kt pieces so the
                # first matmul only waits for its own 64KB
                for kt in range(NKT):
                    nc.gpsimd.dma_start(out=wk[:, kt, :], in_=WkT[:, kt * 256:(kt + 1) * 256])
                for kt in range(NKT):
                    nc.gpsimd.dma_start(out=wv[:, kt, :], in_=WvT[:, kt * 256:(kt + 1) * 256])
                for kt in range(NKT):
                    nc.gpsimd.dma_start(out=wq[:, kt, :], in_=WqT[:, kt * 256:(kt + 1) * 256])
                for h in range(HPC):
                    nc.gpsimd.dma_start(out=wo[h][:], in_=WoT[h * HD:(h + 1) * HD, :])

                def rope_phase1(ps, cs, out_sl, w, swp_pool):
                    # rotate-copies on ACT + the single DVE op that reads the
                    # PSUM tile; after this the PSUM slot is free
                    swp = swp_pool.tile([128, PAIR], f32, name="swp", tag="rt", bufs=4)
                    nc.scalar.copy(swp[0:64, :w], ps[64:128, :w])
                    nc.scalar.copy(swp[64:128, :w], ps[0:64, :w])
                    nc.vector.tensor_mul(out_sl, ps[:, :w], cs[:, :w])
                    return swp

                def rope_phase2(swp, sn, out_sl, w):
                    nc.vector.tensor_mul(swp[:, :w], swp[:, :w], sn[:, :w])
                    nc.vector.tensor_add(out_sl, out_sl, swp[:, :w])

                # ------------- projections (kv first, then q) -------------
                with tc.tile_pool(name="xpool", bufs=1) as xp, \
                     tc.tile_pool(name="rtP", bufs=3) as rtp:
                    xm = [xp.tile([128, S], bf16, name=f"xm{kt}", uniquify=True)
                          for kt in range(NKT)]
                    xq = [xp.tile([128, T], bf16, name=f"xq{kt}", uniquify=True)
                          for kt in range(NKT)]

                    def emit_qcp(c0, cw):
                        cs_sb = csp.tile([128, PAIR], bf16, name="cos_sb", tag="cos", bufs=3)
                        sn_sb = csp.tile([128, PAIR], bf16, name="sin_sb", tag="sin", bufs=3)
                        nc.sync.dma_start(out=cs_sb[:, :cw], in_=cosq[:, c0:c0 + cw])
                        nc.sync.dma_start(out=sn_sb[:, :cw], in_=sinq[:, c0:c0 + cw])
                        for kt in range(NKT):
                            nc.sync.dma_start(out=xq[kt][:, c0:c0 + cw],
                                              in_=xT[kt * 128:(kt + 1) * 128, c0:c0 + cw])
                        qps = [psA.tile([128, PAIR], f32, name="qps", tag="sc", bufs=2)
                               for _ in range(HPC)]
                        halves = [(o, min(CW, cw - o)) for o in range(0, cw, CW)]
                        for kt in range(NKT):
                            for h in range(HPC):
                                for (o, ow) in halves:
                                    nc.tensor.matmul(
                                        qps[h][:, o:o + ow],
                                        wq[:, kt, h * HD:(h + 1) * HD],
                                        xq[kt][:, c0 + o:c0 + o + ow],
                                        start=(kt == 0), stop=(kt == NKT - 1),
                                        skip_group_check=True)
                        swps = [rope_phase1(qps[h], cs_sb, qT_r[h][:, c0:c0 + cw], cw, rtp)
                                for h in range(HPC)]
                        for h in range(HPC):
                            rope_phase2(swps[h], sn_sb, qT_r[h][:, c0:c0 + cw], cw)

                    def emit_scp(s0, sw):
                        cs_sb = csp.tile([128, PAIR], bf16, name="cos_sb", tag="cos", bufs=3)
                        sn_sb = csp.tile([128, PAIR], bf16, name="sin_sb", tag="sin", bufs=3)
                        nc.sync.dma_start(out=cs_sb[:, :sw], in_=cosk[:, s0:s0 + sw])
                        nc.sync.dma_start(out=sn_sb[:, :sw], in_=sink[:, s0:s0 + sw])
                        # chunk-major DMA: all kt pieces of this column range
                        for kt in range(NKT):
                            nc.sync.dma_start(out=xm[kt][:, s0:s0 + sw],
                                              in_=xmelT[kt * 128:(kt + 1) * 128, s0:s0 + sw])
                        # k proj, kt-outer so the PE starts on the first piece
                        kps = [psA.tile([128, PAIR], f32, name="kps", tag="sc", bufs=2)
                               for _ in range(HPC)]
                        halves = [(o, min(CW, sw - o)) for o in range(0, sw, CW)]
                        for kt in range(NKT):
                            for h in range(HPC):
                                for (o, ow) in halves:
                                    nc.tensor.matmul(
                                        kps[h][:, o:o + ow],
                                        wk[:, kt, h * HD:(h + 1) * HD],
                                        xm[kt][:, s0 + o:s0 + o + ow],
                                        start=(kt == 0), stop=(kt == NKT - 1),
                                        skip_group_check=True)
                        swps = [rope_phase1(kps[h], cs_sb, kT_r[h][:, s0:s0 + sw], sw, rtp)
                                for h in range(HPC)]
                        for h in range(HPC):
                            rope_phase2(swps[h], sn_sb, kT_r[h][:, s0:s0 + sw], sw)
                        # v proj for the st tiles of this column range
                        for st in range(s0 // 128, _ceil_div(s0 + sw, 128)):
                            t0 = st * 128
                            scnt = min(128, S - t0)
                            vps = psB.tile([128, HPC * HD], f32, name="vps", tag="acc", bufs=2)
                            for kt in range(NKT):
                                nc.tensor.matmul(
                                    vps[:scnt, :],
                                    xm[kt][:, t0:t0 + scnt],
                                    wv[:, kt, :],
                                    start=(kt == 0), stop=(kt == NKT - 1))
                            nc.scalar.copy(v_sb[:scnt, st, :], vps[:scnt, :])

                    emit_scp(*s_big[0])
                    emit_qcp(*t_big[0])
                    emit_scp(*s_big[1])
                    emit_qcp(*t_big[1])
                    emit_scp(*s_big[2])

                # ---------------- attention + out projection ----------------
                with tc.tile_pool(name="pP", bufs=n_st + 10) as pP, \
                     tc.tile_pool(name="ppairP", bufs=18) as ppP:
                    G = (n_st + 3) // 4   # pipeline group size (4 groups/block)
                    groups = []
                    for pi in range(len(t_pairs)):
                        for h in range(HPC):
                            for g0 in range(0, n_st, G):
                                groups.append((pi, h, g0, min(G, n_st - g0)))
                    DELAY = 4
                    blocks = {}

                    def emit_sc_exp(key):
                        pi, h, g0, gc = key
                        pair = t_pairs[pi]
                        pw = sum(cw for _, cw in pair)
                        bk = blocks.setdefault((pi, h), {"ptiles": {}, "pairs": {}})
                        for st in range(g0, g0 + gc):
                            s0 = st * 128
                            scnt = min(128, S - s0)
                            scps = psA.tile([128, PAIR], f32, name="scps", tag="sc", bufs=2)
                            for ci, (c0, cw) in enumerate(pair):
                                nc.tensor.matmul(
                                    scps[:scnt, ci * CW: ci * CW + cw],
                                    kT_r[h][:, s0:s0 + scnt],
                                    qT_r[h][:, c0:c0 + cw],
                                    start=True, stop=True,
                                    skip_group_check=True)
                            p_t = pP.tile([128, PAIR], bf16, name="p_t", tag="p", bufs=n_st + 10)
                            nc.scalar.activation(p_t[:scnt, :pw], scps[:scnt, :pw],
                                                 mybir.ActivationFunctionType.Exp)
                            bk["ptiles"][st] = (p_t, scnt)
                            # chunked p sums (DVE, bf16 2x) feed the Z matmuls
                            if st < n_full:
                                if st % 4 == 0:
                                    bk["zacc"] = p_t
                                else:
                                    pp_t = ppP.tile([128, PAIR], bf16, name="pp_t", tag="pp", bufs=10)
                                    nc.vector.tensor_add(pp_t[:, :pw], bk["zacc"][:, :pw],
                                                         p_t[:, :pw])
                                    bk["zacc"] = pp_t
                                if st in z_chunk_of_last:
                                    bk["pairs"][z_chunk_of_last[st]] = bk["zacc"]

                    def emit_zav(key):
                        pi, h, g0, gc = key
                        pair = t_pairs[pi]
                        bk = blocks[(pi, h)]
                        last = (g0 + gc == n_st)
                        if g0 == 0:
                            # ci=0 lives at partition 0, ci=1 at partition 32
                            # (matmul out base partition must be 0/32/64)
                            bk["zps"] = psC.tile([64, CW], f32, name="zps", tag="z", bufs=2)
                            bk["o2"] = [psB.tile([128, CW], f32, name="o2ps", tag="acc", bufs=2)
                                        for _ in pair]
                        # Z matmuls on chunk-summed tiles (+ ragged singles at the end)
                        sts = list(range(g0, g0 + gc))
                        for cki, ck in enumerate(z_chunks):
                            if ck[-1] in sts:
                                pp_t = bk["pairs"][cki]
                                first = (cki == 0)
                                for ci, (c0, cw) in enumerate(pair):
                                    nc.tensor.matmul(
                                        bk["zps"][32 * ci:32 * ci + 1, :cw],
                                        ones[:128, :],
                                        pp_t[:, ci * CW: ci * CW + cw],
                                        start=first, stop=False,
                                        skip_group_check=True)
                        if last:
                            for si, st in enumerate(z_singles):
                                p_t, scnt = bk["ptiles"][st]
                                stop = (si == len(z_singles) - 1)
                                for ci, (c0, cw) in enumerate(pair):
                                    nc.tensor.matmul(
                                        bk["zps"][32 * ci:32 * ci + 1, :cw],
                                        ones[:scnt, :],
                                        p_t[:scnt, ci * CW: ci * CW + cw],
                                        start=False, stop=stop,
                                        skip_group_check=True)
                            # stage Z rows to SBUF as bf16 (single cast-copy
                            # each); broadcast + reciprocal happen later
                            bk["zrb"] = []
                            for ci, (c0, cw) in enumerate(pair):
                                zrb = zP.tile([1, CW], bf16, name="zrb", tag="zrb", bufs=4)
                                nc.vector.tensor_copy(zrb[:, :], bk["zps"][32 * ci:32 * ci + 1, :])
                                bk["zrb"].append(zrb)
                        for st in sts:
                            p_t, scnt = bk["ptiles"][st]
                            for ci, (c0, cw) in enumerate(pair):
                                nc.tensor.matmul(
                                    bk["o2"][ci][:, :cw],
                                    v_sb[:scnt, st, h * HD:(h + 1) * HD],
                                    p_t[:scnt, ci * CW: ci * CW + cw],
                                    start=(st == 0), stop=(st == n_st - 1))
                        if last:
                            # broadcast 1/Z across partitions with a ones-column
                            # matmul (no DMA), stage via ACT, then normalize
                            bk["ao"] = []
                            for ci, (c0, cw) in enumerate(pair):
                                zrp = psC.tile([128, CW], f32, name="zrp", tag="z", bufs=2)
                                nc.tensor.matmul(zrp[:, :], onesrow[:, :], bk["zrb"][ci][:, :],
                                                 start=True, stop=True,
                                                 skip_group_check=True)
                                zrep = zP.tile([128, CW], f32, name="zrep", tag="zrep", bufs=4)
                                nc.vector.reciprocal_approx_fast(out=zrep[:, :], in_=zrp[:, :])
                                ao_h = aoP.tile([128, CW], bf16, name="ao", tag="ao", bufs=2 * HPC + 2)
                                nc.vector.tensor_mul(ao_h[:, :cw], bk["o2"][ci][:, :cw],
                                                     zrep[:, :cw])
                                bk["ao"].append(ao_h)

                    def emit_outproj(pi):
                        pair = t_pairs[pi]
                        for ci, (c0, cw) in enumerate(pair):
                            for tt in range(cw // 128):
                                y_sb = yP.tile([128, DIM], f32, name="y_sb", tag="ysb", bufs=2)
                                for nn in range(DIM // 512):
                                    yps = psA.tile([128, 512], f32, name="yps", tag="sc", bufs=2)
                                    for h in range(HPC):
                                        nc.tensor.matmul(
                                            yps[:],
                                            blocks[(pi, h)]["ao"][ci][:, tt * 128:(tt + 1) * 128],
                                            wo[h][:, nn * 512:(nn + 1) * 512],
                                            start=(h == 0), stop=(h == HPC - 1))
                                    nc.vector.tensor_copy(y_sb[:, nn * 512:(nn + 1) * 512], yps[:])
                                nc.sync.dma_start(out=y[c0 + tt * 128: c0 + (tt + 1) * 128, :], in_=y_sb[:])

                    # outproj(pi) is delayed one pipeline slot past its
                    # block-last zav so the 1/Z bounce latency hides under the
                    # next group's matmuls
                    op_after = {}
                    for idx, (pi, h, g0, gc) in enumerate(groups):
                        if g0 + gc == n_st and h == HPC - 1:
                            op_after[idx + 1 if idx + 1 < len(groups) else -1] = pi

                    def do_finish(j):
                        emit_zav(groups[j])
                        if j in op_after:
                            emit_outproj(op_after[j])

                    for i, key in enumerate(groups):
                        emit_sc_exp(key)
                        if i >= DELAY:
                            do_finish(i - DELAY)
                    for j in range(max(0, len(groups) - DELAY), len(groups)):
                        do_finish(j)
                    if -1 in op_after:
                        emit_outproj(op_after[-1])

    nc.compile()
    return nc


def _host_tables(T=T, S=S):
    scale = float(HD) ** (-0.25)
    inv = 1.0 / (ROPE_BASE ** (np.arange(0, HD, 2, dtype=np.float64) / HD))  # [64]

    def tables(L):
        fr = np.outer(inv, np.arange(L, dtype=np.float64))  # [64, L]
        c = np.cos(fr) * scale
        s = np.sin(fr) * scale
        import ml_dtypes
        cos = np.concatenate([c, c], axis=0).astype(ml_dtypes.bfloat16)
        sin = np.concatenate([-s, s], axis=0).astype(ml_dtypes.bfloat16)
        return np.ascontiguousarray(cos), np.ascontiguousarray(sin)

    cosq_, sinq_ = tables(T)
    cosk_, sink_ = tables(S)
    return cosq_, sinq_, cosk_, sink_


def make_in_maps(x, xmel, Wq, Wkv, Wout):
    import ml_dtypes
    bf = ml_dtypes.bfloat16
    Bx, Tx, C = x.shape
    Sx = xmel.shape[1]
    cosq_, sinq_, cosk_, sink_ = _host_tables(Tx, Sx)

    x = np.asarray(x, dtype=np.float32)
    xmel = np.asarray(xmel, dtype=np.float32)
    Wq = np.asarray(Wq, dtype=np.float32)
    Wkv = np.asarray(Wkv, dtype=np.float32)
    Wout = np.asarray(Wout, dtype=np.float32)

    xT_b = [np.ascontiguousarray(x[b].T).astype(bf) for b in range(Bx)]
    xmelT_b = [np.ascontiguousarray(xmel[b].T).astype(bf) for b in range(Bx)]
    gsz = HPC * HD  # 256
    WqT_g, WkT_g, WvT_g, WoT_g = [], [], [], []
    for g in range(NG):
        r0 = g * gsz
        def prearr(wt):  # [DIM, gsz] -> [128, NKT*gsz], row p holds [kt, n]
            return np.ascontiguousarray(
                wt.reshape(NKT, 128, gsz).transpose(1, 0, 2).reshape(128, NKT * gsz)).astype(bf)
        WqT_g.append(prearr(Wq[r0:r0 + gsz, :].T))
        WkT_g.append(prearr(Wkv[r0:r0 + gsz, :].T))
        WvT_g.append(prearr(Wkv[DIM + r0:DIM + r0 + gsz, :].T))
        WoT_g.append(np.ascontiguousarray(Wout[:, r0:r0 + gsz].T).astype(bf))

    in_maps = []
    for c in range(Bx * NG):
        b, g = c // NG, c % NG
        in_maps.append({
            "xT": xT_b[b], "xmelT": xmelT_b[b],
            "WqT": WqT_g[g], "WkT": WkT_g[g], "WvT": WvT_g[g], "WoT": WoT_g[g],
            "cosq": cosq_, "sinq": sinq_, "cosk": cosk_, "sink": sink_,
        })
    return in_maps


def kernel(x, xmel, Wq, Wkv, Wout):
    from concourse.bass_utils import run_bass_kernel_spmd

    x = np.asarray(x, dtype=np.float32)
    xmel = np.asarray(xmel, dtype=np.float32)
    Bx, Tx, C = x.shape
    Sx = xmel.shape[1]
    assert (Bx, Tx, C, Sx) == (B, T, DIM, S)

    if "nc" not in _cache:
        _cache["nc"] = build_nc()
    nc = _cache["nc"]

    in_maps = make_in_maps(x, xmel,
                           np.asarray(Wq, dtype=np.float32),
                           np.asarray(Wkv, dtype=np.float32),
                           np.asarray(Wout, dtype=np.float32))
    res = run_bass_kernel_spmd(nc, in_maps, list(range(8)))
    out = np.zeros((B, T, DIM), dtype=np.float32)
    for c in range(8):
        b = c // NG
        out[b] += np.asarray(res.results[c]["y"], dtype=np.float32)
    return out
